# revision 10
# baseline (speedup 1.0000x reference)
"""Trainium2 Bass kernel for nn_Encoder_20426864460432 (gnn_message_passing).

Strategy (8 NeuronCores, data-parallel over edges/nodes):
  - Edges are sharded contiguously (core n: edges [n*512K,(n+1)*512K)) and,
    within each core's shard, processed in col-sorted order (the permutation
    is part of the sharding; the host inverse-permutes the per-edge outputs
    while unsharding).
  - x[col] for the sorted stream is run-length EXPANDED at DVE line rate:
    per-partition x windows are block-gathered (one indirect DMA), run-start
    values are placed with GPSIMD local_scatter (int16 hi/lo halves, -1 pads
    ignored), recombined with DVE integer ops, and a tensor_tensor_scan
    (state = state*mask + data1) expands runs.
  - x[row] (random in col-order) is gathered with GPSIMD ap_gather from an
    SBUF-resident replicated x table ([128, 32768] f32).
  - Edge math (diff, radial, 1/(norm+1), coord_diff) on DVE/ACT; the
    cross-partition component reduction and per-edge scalar broadcast use
    PE matmuls with constant 0/1 comb matrices.
  - Node head: h = concat(charges, emb[cat]) * mask via one-hot(100) PE
    matmul; parameters = h @ W + b via PE (K split 2x128, 128-node tiles).
  - Host work is integer-only sharding/layout prep (argsort, int16 index
    interleave, one-hot/broadcast constants) plus output unpermute/concat.

Layout: sorted rank s in a core's shard maps to (k, j, ST, t):
  s = k*65536 + j*16384 + ST*1024 + t   (k: gpsimd core, j: subsegment)
Edge (k,j,ST,t) components live on SBUF partition 16k+3j+c at free pos t.
"""

import os
import numpy as np

import concourse.bacc as bacc
import concourse.bass as bass
import concourse.mybir as mybir
import concourse.tile as tile
from concourse import bass_utils

B, N, DIM, MAX_Z = 256, 128, 256, 100
V = B * N                  # 32768 nodes
E = B * N * N              # 4194304 edges
N_CORES = 8
E_NC = E // N_CORES        # 524288
V_NC = V // N_CORES        # 4096
P = 128

T = 1024                   # slots per (k,j) per supertile
NJ = 4                     # subsegments per core
N_ST = E_NC // (8 * NJ * T)  # 16 supertiles
NCHK = 2                   # 512-slot scatter chunks per tile
CHK = T // NCHK            # 512
WCAP = 192                 # max node-window span per 512-slot chunk
PAD = WCAP + 32
Vp = V + PAD

F32 = mybir.dt.float32
I16 = mybir.dt.int16
I32 = mybir.dt.int32

_PROGRAM_CACHE = {}


def _install_ntff_shim():
    import sys, types
    if "antenv.axon_hooks" in sys.modules:
        return
    mod = types.ModuleType("antenv.axon_hooks")
    state = {"hook": None}
    mod.set_axon_ntff_profile_hook = lambda h: state.__setitem__("hook", h)
    mod.get_axon_ntff_profile_hook = lambda: state["hook"]
    sys.modules["antenv.axon_hooks"] = mod
    import antenv
    antenv.axon_hooks = mod
    try:
        from trn_agent_boot.trn_boot import _ntff_profile_via_ctypes
        mod.set_axon_ntff_profile_hook(
            _ntff_profile_via_ctypes("/opt/axon/libaxon_pjrt.so")
        )
    except Exception:
        pass


def _build_program():
    if "nc" in _PROGRAM_CACHE:
        return _PROGRAM_CACHE["nc"]

    nc = bacc.Bacc("TRN2", target_bir_lowering=False, debug=False)

    # ---- inputs ----
    xt_d = nc.dram_tensor("xt", [3, V], F32, kind="ExternalInput")
    xtf_d = nc.dram_tensor("xtflat", [3 * Vp], F32, kind="ExternalInput")
    row_d = nc.dram_tensor("rowidx", [N_ST, NJ, P, T // 16], I16, kind="ExternalInput")
    win_d = nc.dram_tensor("win32", [N_ST, P, NCHK], I32, kind="ExternalInput")
    lo_d = nc.dram_tensor("loidx", [N_ST, NCHK, P, WCAP], I16, kind="ExternalInput")
    msk_d = nc.dram_tensor("maskf", [N_ST, P, T], F32, kind="ExternalInput")
    oh_d = nc.dram_tensor("onehotb", [P, V_NC], F32, kind="ExternalInput")
    nm_d = nc.dram_tensor("maskb", [P, V_NC], F32, kind="ExternalInput")
    chg_d = nc.dram_tensor("charges1", [1, V_NC], F32, kind="ExternalInput")
    emb_d = nc.dram_tensor("embp", [P, DIM], F32, kind="ExternalInput")
    w_d = nc.dram_tensor("wpack", [P, 2 * DIM * 2], F32, kind="ExternalInput")
    bb_d = nc.dram_tensor("biasb", [P, 2 * DIM], F32, kind="ExternalInput")
    comb_d = nc.dram_tensor("comb", [P, P], F32, kind="ExternalInput")
    bcst_d = nc.dram_tensor("bcst", [P, P], F32, kind="ExternalInput")
    cvec_d = nc.dram_tensor("cvec", [P, 1], F32, kind="ExternalInput")

    # ---- outputs ----
    par_d = nc.dram_tensor("params", [V_NC, 2 * DIM], F32, kind="ExternalOutput")
    rad_d = nc.dram_tensor("radial", [E_NC], F32, kind="ExternalOutput")
    cd_d = nc.dram_tensor("cdiff", [3, E_NC], F32, kind="ExternalOutput")

    with tile.TileContext(nc) as tc:
        with tc.tile_pool(name="const", bufs=1) as cpool:
            table = cpool.tile([P, V], F32)
            for j16 in range(16):
                nc.sync.dma_start(
                    out=table[:].rearrange("(k j) v -> j k v", j=16)[j16],
                    in_=xt_d.ap()[j16 % 3 : j16 % 3 + 1, :].to_broadcast([8, V]),
                )
            emb_sb = cpool.tile([P, DIM], F32)
            nc.sync.dma_start(out=emb_sb[:], in_=emb_d.ap())
            w_sb = cpool.tile([P, 2 * DIM * 2], F32)
            nc.sync.dma_start(out=w_sb[:], in_=w_d.ap())
            bb_sb = cpool.tile([P, 2 * DIM], F32)
            nc.sync.dma_start(out=bb_sb[:], in_=bb_d.ap())
            comb_sb = cpool.tile([P, P], F32)
            nc.sync.dma_start(out=comb_sb[:], in_=comb_d.ap())
            bcst_sb = cpool.tile([P, P], F32)
            nc.sync.dma_start(out=bcst_sb[:], in_=bcst_d.ap())
            cvec_sb = cpool.tile([P, 1], F32)
            nc.sync.dma_start(out=cvec_sb[:], in_=cvec_d.ap())

            # ---------------- node part ----------------
            with (
                tc.tile_pool(name="node", bufs=2) as npool,
                tc.tile_pool(name="npsum", bufs=2, space="PSUM") as npsum,
            ):
                NCHUNK = 512
                for c0 in range(0, V_NC, NCHUNK):
                    sl = slice(c0, c0 + NCHUNK)
                    oh = npool.tile([P, NCHUNK], F32, tag="oh")
                    nc.sync.dma_start(out=oh[:], in_=oh_d.ap()[:, sl])
                    mk = npool.tile([P, NCHUNK], F32, tag="mk")
                    nc.sync.dma_start(out=mk[:], in_=nm_d.ap()[:, sl])
                    ch = npool.tile([1, NCHUNK], F32, tag="ch")
                    nc.sync.dma_start(out=ch[:], in_=chg_d.ap()[:, sl])

                    ph1 = npsum.tile([P, NCHUNK], F32, tag="ph", space="PSUM")
                    nc.tensor.matmul(out=ph1[:], lhsT=emb_sb[:, 0:P], rhs=oh[:],
                                     start=True, stop=True)
                    nc.vector.tensor_copy(out=ph1[0:1, :], in_=ch[:])
                    ph2 = npsum.tile([P, NCHUNK], F32, tag="ph2", space="PSUM")
                    nc.tensor.matmul(out=ph2[:], lhsT=emb_sb[:, P : 2 * P], rhs=oh[:],
                                     start=True, stop=True)
                    h1 = npool.tile([P, NCHUNK], F32, tag="h1")
                    nc.vector.tensor_mul(out=h1[:], in0=ph1[:], in1=mk[:])
                    h2 = npool.tile([P, NCHUNK], F32, tag="h2")
                    nc.vector.tensor_mul(out=h2[:], in0=ph2[:], in1=mk[:])

                    for s in range(NCHUNK // P):
                        ssl = slice(s * P, (s + 1) * P)
                        pp = npsum.tile([P, 2 * DIM], F32, tag="pp", space="PSUM")
                        nc.tensor.matmul(out=pp[:], lhsT=h1[:, ssl],
                                         rhs=w_sb[:, 0 : 2 * DIM],
                                         start=True, stop=False)
                        nc.tensor.matmul(out=pp[:], lhsT=h2[:, ssl],
                                         rhs=w_sb[:, 2 * DIM :],
                                         start=False, stop=True)
                        po = npool.tile([P, 2 * DIM], F32, tag="po")
                        nc.vector.tensor_add(out=po[:], in0=pp[:], in1=bb_sb[:])
                        nc.sync.dma_start(
                            out=par_d.ap()[c0 + s * P : c0 + (s + 1) * P, :],
                            in_=po[:],
                        )

            # ---------------- edge part ----------------
            with (
                tc.tile_pool(name="edge", bufs=2) as epool,
                tc.tile_pool(name="edge1", bufs=2) as e1pool,
                tc.tile_pool(name="stage", bufs=1) as gpool,
                tc.tile_pool(name="esm", bufs=1) as spool,
                tc.tile_pool(name="epsum", bufs=2, space="PSUM") as epsum,
            ):
                for t in range(N_ST):
                    # ---- xc: window gather + placement + scan ----
                    wint = gpool.tile([P, NCHK], I32, tag="wint")
                    nc.sync.dma_start(out=wint[:], in_=win_d.ap()[t])
                    data1 = e1pool.tile([P, T], F32, tag="data1")
                    for chk in range(NCHK):
                        li = gpool.tile([P, WCAP], I16, tag="li")
                        nc.sync.dma_start(out=li[:], in_=lo_d.ap()[t, chk])
                        win = gpool.tile([P, WCAP], F32, tag="win")
                        nc.gpsimd.indirect_dma_start(
                            out=win[:], out_offset=None,
                            in_=xtf_d.ap()[:, None],
                            in_offset=bass.IndirectOffsetOnAxis(
                                ap=wint[:, chk : chk + 1], axis=0),
                        )
                        wi = win[:].bitcast(I32)
                        tmp = gpool.tile([P, WCAP], I32, tag="tmp")
                        lo16 = gpool.tile([P, WCAP], I16, tag="lo16")
                        hi16 = gpool.tile([P, WCAP], I16, tag="hi16")
                        nc.vector.tensor_scalar(
                            out=tmp[:], in0=wi, scalar1=16, scalar2=16,
                            op0=mybir.AluOpType.logical_shift_left,
                            op1=mybir.AluOpType.arith_shift_right)
                        nc.vector.tensor_copy(out=lo16[:], in_=tmp[:])
                        nc.vector.tensor_scalar(
                            out=tmp[:], in0=wi, scalar1=16, scalar2=None,
                            op0=mybir.AluOpType.arith_shift_right)
                        nc.vector.tensor_copy(out=hi16[:], in_=tmp[:])
                        lost = gpool.tile([P, CHK], I16, tag="lost")
                        hist = gpool.tile([P, CHK], I16, tag="hist")
                        nc.gpsimd.local_scatter(
                            out_ap=lost[:], data_ap=lo16[:], idxs_ap=li[:],
                            channels=P, num_elems=CHK, num_idxs=WCAP)
                        nc.gpsimd.local_scatter(
                            out_ap=hist[:], data_ap=hi16[:], idxs_ap=li[:],
                            channels=P, num_elems=CHK, num_idxs=WCAP)
                        lo32 = gpool.tile([P, CHK], I32, tag="lo32")
                        nc.vector.tensor_copy(out=lo32[:], in_=lost[:])
                        nc.vector.tensor_scalar(
                            out=lo32[:], in0=lo32[:], scalar1=0xFFFF, scalar2=None,
                            op0=mybir.AluOpType.bitwise_and)
                        hi32 = gpool.tile([P, CHK], I32, tag="hi32")
                        nc.vector.tensor_copy(out=hi32[:], in_=hist[:])
                        nc.vector.tensor_scalar(
                            out=hi32[:], in0=hi32[:], scalar1=16, scalar2=None,
                            op0=mybir.AluOpType.logical_shift_left)
                        nc.vector.tensor_tensor(
                            out=hi32[:], in0=hi32[:], in1=lo32[:],
                            op=mybir.AluOpType.bitwise_or)
                        nc.vector.tensor_copy(
                            out=data1[:, chk * CHK : (chk + 1) * CHK],
                            in_=hi32[:].bitcast(F32))
                    mskt = e1pool.tile([P, T], F32, tag="mskt")
                    nc.sync.dma_start(out=mskt[:], in_=msk_d.ap()[t])
                    xc = epool.tile([P, T], F32, tag="xc")
                    nc.vector.tensor_tensor_scan(
                        out=xc[:], data0=mskt[:], data1=data1[:], initial=0.0,
                        op0=mybir.AluOpType.mult, op1=mybir.AluOpType.add)

                    # ---- xr gathers (per j) + math ----
                    for j in range(NJ):
                        ri = gpool.tile([P, T // 16], I16, tag="ri")
                        nc.sync.dma_start(out=ri[:], in_=row_d.ap()[t, j])
                        gr = epool.tile([P, T], F32, tag="gr")
                        nc.gpsimd.ap_gather(
                            out_ap=gr[:], in_ap=table[:], idxs_ap=ri[:],
                            channels=P, num_elems=V, d=1, num_idxs=T)
                        # diff (into gr), sq
                        nc.vector.tensor_tensor(
                            out=gr[:], in0=gr[:], in1=xc[:],
                            op=mybir.AluOpType.subtract)
                        sq = epool.tile([P, T], F32, tag="sq")
                        nc.vector.tensor_tensor(
                            out=sq[:], in0=gr[:], in1=gr[:],
                            op=mybir.AluOpType.mult)
                        for s in range(T // 512):
                            ssl = slice(s * 512, (s + 1) * 512)
                            pr = epsum.tile([P, 512], F32, tag="pr", space="PSUM")
                            nc.tensor.matmul(out=pr[:], lhsT=comb_sb[:],
                                             rhs=sq[:, ssl], start=True, stop=True)
                            rsb = spool.tile([P, 512], F32, tag="rsb")
                            nc.scalar.activation(
                                out=rsb[:], in_=pr[:],
                                func=mybir.ActivationFunctionType.Copy, bias=0.0)
                            # radial out: rows 16k+3j; dst rank offset
                            off = j * (N_ST * T) + t * T + s * 512
                            nc.sync.dma_start(
                                out=rad_d.ap()
                                .rearrange("(k r) -> k r", k=8)[:, off : off + 512],
                                in_=rsb[:]
                                .rearrange("(k r) i -> k r i", r=16)[:, 3 * j, :],
                            )
                            nrm = spool.tile([P, 512], F32, tag="nrm")
                            nc.scalar.activation(
                                out=nrm[:], in_=rsb[:],
                                func=mybir.ActivationFunctionType.Sqrt,
                                bias=cvec_sb[:, 0:1])
                            nc.scalar.activation(
                                out=nrm[:], in_=nrm[:],
                                func=mybir.ActivationFunctionType.Copy, bias=1.0)
                            rcp = spool.tile([P, 512], F32, tag="rcp")
                            nc.vector.reciprocal(out=rcp[:], in_=nrm[:])
                            pb = epsum.tile([P, 512], F32, tag="pb", space="PSUM")
                            nc.tensor.matmul(out=pb[:], lhsT=bcst_sb[:], rhs=rcp[:],
                                             start=True, stop=True)
                            nc.vector.tensor_tensor(
                                out=gr[:, ssl], in0=gr[:, ssl], in1=pb[:],
                                op=mybir.AluOpType.mult)
                        # cd out, comp-major
                        for cc in range(3):
                            off = j * (N_ST * T) + t * T
                            nc.sync.dma_start(
                                out=cd_d.ap()[cc]
                                .rearrange("(k r) -> k r", k=8)[:, off : off + T],
                                in_=gr[:]
                                .rearrange("(k r) i -> k r i", r=16)[:, 3 * j + cc, :],
                            )

    nc.compile()
    _PROGRAM_CACHE["nc"] = nc
    return nc


def _prep_edge_shard(rows, cols):
    """Integer-only layout prep for one core's edge shard (col-sorted)."""
    sigma = np.argsort(cols, kind="stable")
    col_s = cols[sigma]
    row_s = rows[sigma]

    # s = k*65536 + j*16384 + ST*1024 + t
    C = col_s.reshape(8, NJ, N_ST, T)
    R = row_s.reshape(8, NJ, N_ST, T)

    # ap_gather row indices: idx[ST, j, 16k+p', s'] = R[k, j, ST, s'*16+p']
    ridx = (
        R.reshape(8, NJ, N_ST, T // 16, 16)
        .transpose(2, 1, 0, 4, 3)  # [ST, j, k, p', s']
        .reshape(N_ST, NJ, P, T // 16)
        .astype(np.int16)
    )

    # window chunks
    Cc = C.reshape(8, NJ, N_ST, NCHK, CHK)
    w = Cc[..., 0].astype(np.int64)            # [k, j, ST, chunk]
    span = Cc[..., -1].astype(np.int64) - w + 1
    assert span.max() <= WCAP, f"window span {span.max()} > WCAP {WCAP}"
    delta = (Cc - w[..., None]).astype(np.int64)  # [k,j,ST,chunk,CHK]

    fo = np.full((8, NJ, N_ST, NCHK, WCAP), -1, dtype=np.int16)
    vals = np.arange(CHK - 1, -1, -1, dtype=np.int16)
    vals = np.broadcast_to(vals, delta.shape)
    np.put_along_axis(fo, delta[..., ::-1], vals, axis=-1)
    # chunk-1 continuation: suppress entry 0 if same col as prev chunk's last
    cont = Cc[..., 1, 0] == Cc[..., 0, -1]     # [k, j, ST]
    f1 = fo[..., 1, 0]
    f1[cont] = -1
    fo[..., 1, 0] = f1

    # mask (run continuation within the supertile segment)
    M = np.zeros((8, NJ, N_ST, T), dtype=np.float32)
    M[..., 1:] = (C[..., 1:] == C[..., :-1]).astype(np.float32)

    # scatter to channel layout ch = 16k + 3j + c
    win32 = np.zeros((N_ST, P, NCHK), dtype=np.int32)
    loidx = np.full((N_ST, NCHK, P, WCAP), -1, dtype=np.int16)
    maskf = np.zeros((N_ST, P, T), dtype=np.float32)
    for chn in range(P):
        k, r = divmod(chn, 16)
        if r >= 12:
            continue
        j, c = divmod(r, 3)
        win32[:, chn, :] = (c * Vp + w[k, j]).astype(np.int32)
        loidx[:, :, chn, :] = fo[k, j]
        maskf[:, chn, :] = M[k, j]
    return sigma, ridx, win32, loidx, maskf


def _prep_inputs(x, categories, charges, edges, node_mask, edge_mask,
                 emb_table, W, b_lin):
    x = np.asarray(x, dtype=np.float32).reshape(V, 3)
    categories = np.asarray(categories).reshape(V)
    charges = np.asarray(charges, dtype=np.float32).reshape(V)
    edges = np.asarray(edges).reshape(2, E)
    node_mask = np.asarray(node_mask, dtype=np.float32).reshape(V)
    emb_table = np.asarray(emb_table, dtype=np.float32)
    W = np.asarray(W, dtype=np.float32)
    b_lin = np.asarray(b_lin, dtype=np.float32)

    assert edges.min() >= 0 and edges.max() < V

    xt = np.ascontiguousarray(x.T)  # [3, V]
    xtflat = np.zeros(3 * Vp, dtype=np.float32)
    for c in range(3):
        xtflat[c * Vp : c * Vp + V] = x[:, c]

    embp = np.zeros((P, DIM), dtype=np.float32)
    embp[:MAX_Z, 1:] = emb_table
    wpack = np.concatenate([W[:P], W[P:]], axis=1)
    biasb = np.ascontiguousarray(
        np.broadcast_to(b_lin[None, :], (P, 2 * DIM)), dtype=np.float32)
    comb = np.zeros((P, P), dtype=np.float32)
    bcst = np.zeros((P, P), dtype=np.float32)
    for k in range(8):
        for j in range(NJ):
            for c in range(3):
                comb[16 * k + 3 * j + c, 16 * k + 3 * j] = 1.0
                bcst[16 * k + 3 * j, 16 * k + 3 * j + c] = 1.0

    in_maps = []
    sigmas = []
    for n in range(N_CORES):
        ebase = n * E_NC
        nbase = n * V_NC
        rows = edges[0, ebase : ebase + E_NC]
        cols = edges[1, ebase : ebase + E_NC]
        sigma, ridx, win32, loidx, maskf = _prep_edge_shard(rows, cols)
        sigmas.append(sigma)

        cats = categories[nbase : nbase + V_NC]
        onehotb = (cats[None, :] == np.arange(P)[:, None]).astype(np.float32)
        maskb = np.ascontiguousarray(
            np.broadcast_to(node_mask[None, nbase : nbase + V_NC], (P, V_NC)),
            dtype=np.float32)
        in_maps.append({
            "xt": xt,
            "xtflat": xtflat,
            "rowidx": ridx,
            "win32": win32,
            "loidx": loidx,
            "maskf": maskf,
            "onehotb": onehotb,
            "maskb": maskb,
            "charges1": charges[None, nbase : nbase + V_NC],
            "embp": embp,
            "wpack": wpack,
            "biasb": biasb,
            "comb": comb,
            "bcst": bcst,
            "cvec": np.full((P, 1), 1e-8, dtype=np.float32),
        })
    return in_maps, sigmas


LAST_EXEC_NS = None


def kernel(**inputs):
    global LAST_EXEC_NS
    trace = bool(os.environ.get("BASS_KERNEL_TRACE"))
    if trace:
        _install_ntff_shim()
    nc = _build_program()
    in_maps, sigmas = _prep_inputs(**inputs)
    res = bass_utils.run_bass_kernel_spmd(
        nc, in_maps, core_ids=list(range(N_CORES)), trace=trace
    )
    LAST_EXEC_NS = res.exec_time_ns

    params = np.concatenate(
        [res.results[n]["params"] for n in range(N_CORES)], axis=0)
    radial = np.empty(E, dtype=np.float32)
    cdiff = np.empty((E, 3), dtype=np.float32)
    for n in range(N_CORES):
        sl = slice(n * E_NC, (n + 1) * E_NC)
        rs = res.results[n]["radial"]     # sorted (rank) order
        cs = res.results[n]["cdiff"]      # [3, E_NC] sorted order
        sigma = sigmas[n]
        radial[sl.start + sigma] = rs
        cdiff[sl.start + sigma, :] = cs.T
    return params, radial.reshape(E, 1), cdiff


# revision 11
# speedup vs baseline: 1.0581x; 1.0581x over previous
"""Trainium2 Bass kernel for nn_Encoder_20426864460432 (gnn_message_passing).

Strategy (8 NeuronCores, data-parallel over edges/nodes):
  - Edges are sharded contiguously (core n: edges [n*512K,(n+1)*512K)) and,
    within each core's shard, processed in col-sorted order (the permutation
    is part of the sharding; the host inverse-permutes the per-edge outputs
    while unsharding).
  - x[col] for the sorted stream is run-length EXPANDED at DVE line rate:
    per-partition x windows are block-gathered (one indirect DMA), run-start
    values are placed with GPSIMD local_scatter (int16 hi/lo halves, -1 pads
    ignored), recombined with DVE integer ops, and a tensor_tensor_scan
    (state = state*mask + data1) expands runs.
  - x[row] (random in col-order) is gathered with GPSIMD ap_gather from an
    SBUF-resident replicated x table ([128, 32768] f32).
  - Edge math (diff, radial, 1/(norm+1), coord_diff) on DVE/ACT; the
    cross-partition component reduction and per-edge scalar broadcast use
    PE matmuls with constant 0/1 comb matrices.
  - Node head: h = concat(charges, emb[cat]) * mask via one-hot(100) PE
    matmul; parameters = h @ W + b via PE (K split 2x128, 128-node tiles).
  - Host work is integer-only sharding/layout prep (argsort, int16 index
    interleave, one-hot/broadcast constants) plus output unpermute/concat.

Layout: sorted rank s in a core's shard maps to (k, j, ST, t):
  s = k*65536 + j*16384 + ST*1024 + t   (k: gpsimd core, j: subsegment)
Edge (k,j,ST,t) components live on SBUF partition 16k+3j+c at free pos t.
"""

import os
import numpy as np

import concourse.bacc as bacc
import concourse.bass as bass
import concourse.mybir as mybir
import concourse.tile as tile
from concourse import bass_utils

B, N, DIM, MAX_Z = 256, 128, 256, 100
V = B * N                  # 32768 nodes
E = B * N * N              # 4194304 edges
N_CORES = 8
E_NC = E // N_CORES        # 524288
V_NC = V // N_CORES        # 4096
P = 128

T = 1024                   # slots per (k,j) per supertile
NJ = 4                     # subsegments per core
N_ST = E_NC // (8 * NJ * T)  # 16 supertiles
NCHK = 2                   # 512-slot scatter chunks per tile
CHK = T // NCHK            # 512
WCAP = 192                 # max node-window span per 512-slot chunk
PAD = WCAP + 32
Vp = V + PAD

F32 = mybir.dt.float32
I16 = mybir.dt.int16
I32 = mybir.dt.int32

_PROGRAM_CACHE = {}


def _install_ntff_shim():
    import sys, types
    if "antenv.axon_hooks" in sys.modules:
        return
    mod = types.ModuleType("antenv.axon_hooks")
    state = {"hook": None}
    mod.set_axon_ntff_profile_hook = lambda h: state.__setitem__("hook", h)
    mod.get_axon_ntff_profile_hook = lambda: state["hook"]
    sys.modules["antenv.axon_hooks"] = mod
    import antenv
    antenv.axon_hooks = mod
    try:
        from trn_agent_boot.trn_boot import _ntff_profile_via_ctypes
        mod.set_axon_ntff_profile_hook(
            _ntff_profile_via_ctypes("/opt/axon/libaxon_pjrt.so")
        )
    except Exception:
        pass


def _build_program():
    if "nc" in _PROGRAM_CACHE:
        return _PROGRAM_CACHE["nc"]

    nc = bacc.Bacc("TRN2", target_bir_lowering=False, debug=False)

    # ---- inputs ----
    xt_d = nc.dram_tensor("xt", [3, V], F32, kind="ExternalInput")
    xtf_d = nc.dram_tensor("xtflat", [3 * Vp], F32, kind="ExternalInput")
    row_d = nc.dram_tensor("rowidx", [N_ST, NJ, P, T // 16], I16, kind="ExternalInput")
    win_d = nc.dram_tensor("win32", [N_ST, P, NCHK], I32, kind="ExternalInput")
    lo_d = nc.dram_tensor("loidx", [N_ST, NCHK, P, WCAP], I16, kind="ExternalInput")
    msk_d = nc.dram_tensor("maskf", [N_ST, P, T], F32, kind="ExternalInput")
    oh_d = nc.dram_tensor("onehotb", [P, V_NC], F32, kind="ExternalInput")
    nm_d = nc.dram_tensor("maskb", [P, V_NC], F32, kind="ExternalInput")
    chg_d = nc.dram_tensor("charges1", [1, V_NC], F32, kind="ExternalInput")
    emb_d = nc.dram_tensor("embp", [P, DIM], F32, kind="ExternalInput")
    w_d = nc.dram_tensor("wpack", [P, 2 * DIM * 2], F32, kind="ExternalInput")
    bb_d = nc.dram_tensor("biasb", [P, 2 * DIM], F32, kind="ExternalInput")
    comb_d = nc.dram_tensor("comb", [P, P], F32, kind="ExternalInput")
    bcst_d = nc.dram_tensor("bcst", [P, P], F32, kind="ExternalInput")
    cvec_d = nc.dram_tensor("cvec", [P, 1], F32, kind="ExternalInput")

    # ---- outputs ----
    par_d = nc.dram_tensor("params", [V_NC, 2 * DIM], F32, kind="ExternalOutput")
    rad_d = nc.dram_tensor("radial", [E_NC], F32, kind="ExternalOutput")
    cd_d = nc.dram_tensor("cdiff", [3, E_NC], F32, kind="ExternalOutput")

    with tile.TileContext(nc) as tc:
        with tc.tile_pool(name="const", bufs=1) as cpool:
            table = cpool.tile([P, V], F32)
            for j16 in range(16):
                nc.sync.dma_start(
                    out=table[:].rearrange("(k j) v -> j k v", j=16)[j16],
                    in_=xt_d.ap()[j16 % 3 : j16 % 3 + 1, :].to_broadcast([8, V]),
                )
            emb_sb = cpool.tile([P, DIM], F32)
            nc.sync.dma_start(out=emb_sb[:], in_=emb_d.ap())
            w_sb = cpool.tile([P, 2 * DIM * 2], F32)
            nc.sync.dma_start(out=w_sb[:], in_=w_d.ap())
            bb_sb = cpool.tile([P, 2 * DIM], F32)
            nc.sync.dma_start(out=bb_sb[:], in_=bb_d.ap())
            comb_sb = cpool.tile([P, P], F32)
            nc.sync.dma_start(out=comb_sb[:], in_=comb_d.ap())
            bcst_sb = cpool.tile([P, P], F32)
            nc.sync.dma_start(out=bcst_sb[:], in_=bcst_d.ap())
            cvec_sb = cpool.tile([P, 1], F32)
            nc.sync.dma_start(out=cvec_sb[:], in_=cvec_d.ap())

            # ---------------- node part ----------------
            with (
                tc.tile_pool(name="node", bufs=2) as npool,
                tc.tile_pool(name="npsum", bufs=2, space="PSUM") as npsum,
            ):
                NCHUNK = 512
                for c0 in range(0, V_NC, NCHUNK):
                    sl = slice(c0, c0 + NCHUNK)
                    oh = npool.tile([P, NCHUNK], F32, tag="oh")
                    nc.sync.dma_start(out=oh[:], in_=oh_d.ap()[:, sl])
                    mk = npool.tile([P, NCHUNK], F32, tag="mk")
                    nc.sync.dma_start(out=mk[:], in_=nm_d.ap()[:, sl])
                    ch = npool.tile([1, NCHUNK], F32, tag="ch")
                    nc.sync.dma_start(out=ch[:], in_=chg_d.ap()[:, sl])

                    ph1 = npsum.tile([P, NCHUNK], F32, tag="ph", space="PSUM")
                    nc.tensor.matmul(out=ph1[:], lhsT=emb_sb[:, 0:P], rhs=oh[:],
                                     start=True, stop=True)
                    nc.vector.tensor_copy(out=ph1[0:1, :], in_=ch[:])
                    ph2 = npsum.tile([P, NCHUNK], F32, tag="ph2", space="PSUM")
                    nc.tensor.matmul(out=ph2[:], lhsT=emb_sb[:, P : 2 * P], rhs=oh[:],
                                     start=True, stop=True)
                    h1 = npool.tile([P, NCHUNK], F32, tag="h1")
                    nc.vector.tensor_mul(out=h1[:], in0=ph1[:], in1=mk[:])
                    h2 = npool.tile([P, NCHUNK], F32, tag="h2")
                    nc.vector.tensor_mul(out=h2[:], in0=ph2[:], in1=mk[:])

                    for s in range(NCHUNK // P):
                        ssl = slice(s * P, (s + 1) * P)
                        pp = npsum.tile([P, 2 * DIM], F32, tag="pp", space="PSUM")
                        nc.tensor.matmul(out=pp[:], lhsT=h1[:, ssl],
                                         rhs=w_sb[:, 0 : 2 * DIM],
                                         start=True, stop=False)
                        nc.tensor.matmul(out=pp[:], lhsT=h2[:, ssl],
                                         rhs=w_sb[:, 2 * DIM :],
                                         start=False, stop=True)
                        po = npool.tile([P, 2 * DIM], F32, tag="po")
                        nc.vector.tensor_add(out=po[:], in0=pp[:], in1=bb_sb[:])
                        nc.sync.dma_start(
                            out=par_d.ap()[c0 + s * P : c0 + (s + 1) * P, :],
                            in_=po[:],
                        )

            # ---------------- edge part ----------------
            with (
                tc.tile_pool(name="edge", bufs=2) as epool,
                tc.tile_pool(name="edge1", bufs=2) as e1pool,
                tc.tile_pool(name="stage", bufs=2) as gpool,
                tc.tile_pool(name="esm", bufs=1) as spool,
                tc.tile_pool(name="epsum", bufs=2, space="PSUM") as epsum,
            ):
                for t in range(N_ST):
                    # ---- xc: window gather + placement + scan ----
                    wint = gpool.tile([P, NCHK], I32, tag="wint")
                    nc.sync.dma_start(out=wint[:], in_=win_d.ap()[t])
                    data1 = e1pool.tile([P, T], F32, tag="data1")
                    for chk in range(NCHK):
                        li = gpool.tile([P, WCAP], I16, tag="li")
                        nc.sync.dma_start(out=li[:], in_=lo_d.ap()[t, chk])
                        win = gpool.tile([P, WCAP], F32, tag="win")
                        nc.gpsimd.indirect_dma_start(
                            out=win[:], out_offset=None,
                            in_=xtf_d.ap()[:, None],
                            in_offset=bass.IndirectOffsetOnAxis(
                                ap=wint[:, chk : chk + 1], axis=0),
                        )
                        wi = win[:].bitcast(I32)
                        tmp = gpool.tile([P, WCAP], I32, tag="tmp")
                        lo16 = gpool.tile([P, WCAP], I16, tag="lo16")
                        hi16 = gpool.tile([P, WCAP], I16, tag="hi16")
                        nc.vector.tensor_scalar(
                            out=tmp[:], in0=wi, scalar1=16, scalar2=16,
                            op0=mybir.AluOpType.logical_shift_left,
                            op1=mybir.AluOpType.arith_shift_right)
                        nc.vector.tensor_copy(out=lo16[:], in_=tmp[:])
                        nc.vector.tensor_scalar(
                            out=tmp[:], in0=wi, scalar1=16, scalar2=None,
                            op0=mybir.AluOpType.arith_shift_right)
                        nc.vector.tensor_copy(out=hi16[:], in_=tmp[:])
                        lost = gpool.tile([P, CHK], I16, tag="lost")
                        hist = gpool.tile([P, CHK], I16, tag="hist")
                        nc.gpsimd.local_scatter(
                            out_ap=lost[:], data_ap=lo16[:], idxs_ap=li[:],
                            channels=P, num_elems=CHK, num_idxs=WCAP)
                        nc.gpsimd.local_scatter(
                            out_ap=hist[:], data_ap=hi16[:], idxs_ap=li[:],
                            channels=P, num_elems=CHK, num_idxs=WCAP)
                        lo32 = gpool.tile([P, CHK], I32, tag="lo32")
                        nc.vector.tensor_copy(out=lo32[:], in_=lost[:])
                        nc.vector.tensor_scalar(
                            out=lo32[:], in0=lo32[:], scalar1=0xFFFF, scalar2=None,
                            op0=mybir.AluOpType.bitwise_and)
                        hi32 = gpool.tile([P, CHK], I32, tag="hi32")
                        nc.vector.tensor_copy(out=hi32[:], in_=hist[:])
                        nc.vector.tensor_scalar(
                            out=hi32[:], in0=hi32[:], scalar1=16, scalar2=None,
                            op0=mybir.AluOpType.logical_shift_left)
                        nc.vector.tensor_tensor(
                            out=hi32[:], in0=hi32[:], in1=lo32[:],
                            op=mybir.AluOpType.bitwise_or)
                        nc.vector.tensor_copy(
                            out=data1[:, chk * CHK : (chk + 1) * CHK],
                            in_=hi32[:].bitcast(F32))
                    mskt = e1pool.tile([P, T], F32, tag="mskt")
                    nc.sync.dma_start(out=mskt[:], in_=msk_d.ap()[t])
                    xc = epool.tile([P, T], F32, tag="xc")
                    nc.vector.tensor_tensor_scan(
                        out=xc[:], data0=mskt[:], data1=data1[:], initial=0.0,
                        op0=mybir.AluOpType.mult, op1=mybir.AluOpType.add)

                    # ---- xr gathers (per j) + math ----
                    for j in range(NJ):
                        ri = gpool.tile([P, T // 16], I16, tag="ri")
                        nc.sync.dma_start(out=ri[:], in_=row_d.ap()[t, j])
                        gr = epool.tile([P, T], F32, tag="gr")
                        nc.gpsimd.ap_gather(
                            out_ap=gr[:], in_ap=table[:], idxs_ap=ri[:],
                            channels=P, num_elems=V, d=1, num_idxs=T)
                        # diff (into gr), sq
                        nc.vector.tensor_tensor(
                            out=gr[:], in0=gr[:], in1=xc[:],
                            op=mybir.AluOpType.subtract)
                        sq = epool.tile([P, T], F32, tag="sq")
                        nc.vector.tensor_tensor(
                            out=sq[:], in0=gr[:], in1=gr[:],
                            op=mybir.AluOpType.mult)
                        for s in range(T // 512):
                            ssl = slice(s * 512, (s + 1) * 512)
                            pr = epsum.tile([P, 512], F32, tag="pr", space="PSUM")
                            nc.tensor.matmul(out=pr[:], lhsT=comb_sb[:],
                                             rhs=sq[:, ssl], start=True, stop=True)
                            rsb = spool.tile([P, 512], F32, tag="rsb")
                            nc.scalar.activation(
                                out=rsb[:], in_=pr[:],
                                func=mybir.ActivationFunctionType.Copy, bias=0.0)
                            # radial out: rows 16k+3j; dst rank offset
                            off = j * (N_ST * T) + t * T + s * 512
                            nc.scalar.dma_start(
                                out=rad_d.ap()
                                .rearrange("(k r) -> k r", k=8)[:, off : off + 512],
                                in_=rsb[:]
                                .rearrange("(k r) i -> k r i", r=16)[:, 3 * j, :],
                            )
                            nrm = spool.tile([P, 512], F32, tag="nrm")
                            nc.scalar.activation(
                                out=nrm[:], in_=rsb[:],
                                func=mybir.ActivationFunctionType.Sqrt,
                                bias=cvec_sb[:, 0:1])
                            nc.scalar.activation(
                                out=nrm[:], in_=nrm[:],
                                func=mybir.ActivationFunctionType.Copy, bias=1.0)
                            rcp = spool.tile([P, 512], F32, tag="rcp")
                            nc.vector.reciprocal(out=rcp[:], in_=nrm[:])
                            pb = epsum.tile([P, 512], F32, tag="pb", space="PSUM")
                            nc.tensor.matmul(out=pb[:], lhsT=bcst_sb[:], rhs=rcp[:],
                                             start=True, stop=True)
                            nc.vector.tensor_tensor(
                                out=gr[:, ssl], in0=gr[:, ssl], in1=pb[:],
                                op=mybir.AluOpType.mult)
                        # cd out, comp-major
                        for cc in range(3):
                            off = j * (N_ST * T) + t * T
                            nc.scalar.dma_start(
                                out=cd_d.ap()[cc]
                                .rearrange("(k r) -> k r", k=8)[:, off : off + T],
                                in_=gr[:]
                                .rearrange("(k r) i -> k r i", r=16)[:, 3 * j + cc, :],
                            )

    nc.compile()
    _PROGRAM_CACHE["nc"] = nc
    return nc


def _prep_edge_shard(rows, cols):
    """Integer-only layout prep for one core's edge shard (col-sorted)."""
    sigma = np.argsort(cols, kind="stable")
    col_s = cols[sigma]
    row_s = rows[sigma]

    # s = k*65536 + j*16384 + ST*1024 + t
    C = col_s.reshape(8, NJ, N_ST, T)
    R = row_s.reshape(8, NJ, N_ST, T)

    # ap_gather row indices: idx[ST, j, 16k+p', s'] = R[k, j, ST, s'*16+p']
    ridx = (
        R.reshape(8, NJ, N_ST, T // 16, 16)
        .transpose(2, 1, 0, 4, 3)  # [ST, j, k, p', s']
        .reshape(N_ST, NJ, P, T // 16)
        .astype(np.int16)
    )

    # window chunks
    Cc = C.reshape(8, NJ, N_ST, NCHK, CHK)
    w = Cc[..., 0].astype(np.int64)            # [k, j, ST, chunk]
    span = Cc[..., -1].astype(np.int64) - w + 1
    assert span.max() <= WCAP, f"window span {span.max()} > WCAP {WCAP}"
    delta = (Cc - w[..., None]).astype(np.int64)  # [k,j,ST,chunk,CHK]

    fo = np.full((8, NJ, N_ST, NCHK, WCAP), -1, dtype=np.int16)
    vals = np.arange(CHK - 1, -1, -1, dtype=np.int16)
    vals = np.broadcast_to(vals, delta.shape)
    np.put_along_axis(fo, delta[..., ::-1], vals, axis=-1)
    # chunk-1 continuation: suppress entry 0 if same col as prev chunk's last
    cont = Cc[..., 1, 0] == Cc[..., 0, -1]     # [k, j, ST]
    f1 = fo[..., 1, 0]
    f1[cont] = -1
    fo[..., 1, 0] = f1

    # mask (run continuation within the supertile segment)
    M = np.zeros((8, NJ, N_ST, T), dtype=np.float32)
    M[..., 1:] = (C[..., 1:] == C[..., :-1]).astype(np.float32)

    # scatter to channel layout ch = 16k + 3j + c
    win32 = np.zeros((N_ST, P, NCHK), dtype=np.int32)
    loidx = np.full((N_ST, NCHK, P, WCAP), -1, dtype=np.int16)
    maskf = np.zeros((N_ST, P, T), dtype=np.float32)
    for chn in range(P):
        k, r = divmod(chn, 16)
        if r >= 12:
            continue
        j, c = divmod(r, 3)
        win32[:, chn, :] = (c * Vp + w[k, j]).astype(np.int32)
        loidx[:, :, chn, :] = fo[k, j]
        maskf[:, chn, :] = M[k, j]
    return sigma, ridx, win32, loidx, maskf


def _prep_inputs(x, categories, charges, edges, node_mask, edge_mask,
                 emb_table, W, b_lin):
    x = np.asarray(x, dtype=np.float32).reshape(V, 3)
    categories = np.asarray(categories).reshape(V)
    charges = np.asarray(charges, dtype=np.float32).reshape(V)
    edges = np.asarray(edges).reshape(2, E)
    node_mask = np.asarray(node_mask, dtype=np.float32).reshape(V)
    emb_table = np.asarray(emb_table, dtype=np.float32)
    W = np.asarray(W, dtype=np.float32)
    b_lin = np.asarray(b_lin, dtype=np.float32)

    assert edges.min() >= 0 and edges.max() < V

    xt = np.ascontiguousarray(x.T)  # [3, V]
    xtflat = np.zeros(3 * Vp, dtype=np.float32)
    for c in range(3):
        xtflat[c * Vp : c * Vp + V] = x[:, c]

    embp = np.zeros((P, DIM), dtype=np.float32)
    embp[:MAX_Z, 1:] = emb_table
    wpack = np.concatenate([W[:P], W[P:]], axis=1)
    biasb = np.ascontiguousarray(
        np.broadcast_to(b_lin[None, :], (P, 2 * DIM)), dtype=np.float32)
    comb = np.zeros((P, P), dtype=np.float32)
    bcst = np.zeros((P, P), dtype=np.float32)
    for k in range(8):
        for j in range(NJ):
            for c in range(3):
                comb[16 * k + 3 * j + c, 16 * k + 3 * j] = 1.0
                bcst[16 * k + 3 * j, 16 * k + 3 * j + c] = 1.0

    in_maps = []
    sigmas = []
    for n in range(N_CORES):
        ebase = n * E_NC
        nbase = n * V_NC
        rows = edges[0, ebase : ebase + E_NC]
        cols = edges[1, ebase : ebase + E_NC]
        sigma, ridx, win32, loidx, maskf = _prep_edge_shard(rows, cols)
        sigmas.append(sigma)

        cats = categories[nbase : nbase + V_NC]
        onehotb = (cats[None, :] == np.arange(P)[:, None]).astype(np.float32)
        maskb = np.ascontiguousarray(
            np.broadcast_to(node_mask[None, nbase : nbase + V_NC], (P, V_NC)),
            dtype=np.float32)
        in_maps.append({
            "xt": xt,
            "xtflat": xtflat,
            "rowidx": ridx,
            "win32": win32,
            "loidx": loidx,
            "maskf": maskf,
            "onehotb": onehotb,
            "maskb": maskb,
            "charges1": charges[None, nbase : nbase + V_NC],
            "embp": embp,
            "wpack": wpack,
            "biasb": biasb,
            "comb": comb,
            "bcst": bcst,
            "cvec": np.full((P, 1), 1e-8, dtype=np.float32),
        })
    return in_maps, sigmas


LAST_EXEC_NS = None


def kernel(**inputs):
    global LAST_EXEC_NS
    trace = bool(os.environ.get("BASS_KERNEL_TRACE"))
    if trace:
        _install_ntff_shim()
    nc = _build_program()
    in_maps, sigmas = _prep_inputs(**inputs)
    res = bass_utils.run_bass_kernel_spmd(
        nc, in_maps, core_ids=list(range(N_CORES)), trace=trace
    )
    LAST_EXEC_NS = res.exec_time_ns

    params = np.concatenate(
        [res.results[n]["params"] for n in range(N_CORES)], axis=0)
    radial = np.empty(E, dtype=np.float32)
    cdiff = np.empty((E, 3), dtype=np.float32)
    for n in range(N_CORES):
        sl = slice(n * E_NC, (n + 1) * E_NC)
        rs = res.results[n]["radial"]     # sorted (rank) order
        cs = res.results[n]["cdiff"]      # [3, E_NC] sorted order
        sigma = sigmas[n]
        radial[sl.start + sigma] = rs
        cdiff[sl.start + sigma, :] = cs.T
    return params, radial.reshape(E, 1), cdiff
